# revision 1
# baseline (speedup 1.0000x reference)
"""Single-head causal attention (B=4, T=2048, C=1024, H=64) on 8 TRN2 NeuronCores.

Sharding: each batch b is handled by the core pair (2b, 2b+1). Within a pair,
keys/values are split by interleaved 128-row key-tiles (core parity p owns
global key-tiles {2m+p}).  Every core computes q/k/v projections from its
batch's x (host-supplied pre-transposed, columns permuted own-tiles-first so
the SPMD graph is identical on all cores), then causal scores^T, exp, and the
wei@[v|1] partial sums for ALL queries against ITS OWN keys.  The host adds
the two partial outputs of a pair and normalizes (softmax denominator is the
ones-column of the augmented v matmul).

Math notes:
 - scale = C**-0.5 = 1/32 folded into the exp activation's scale.
 - no max-subtraction: scores*scale ~ N(0, 0.25^2) so exp is tiny/safe.
 - compute in bf16 (fp32 PSUM accumulation); partial sums returned fp32.
"""

import os
import sys

sys.path.insert(0, "/opt/trn_rl_repo")

import numpy as np
import ml_dtypes

B, T, C, H = 4, 2048, 1024, 64
NKT = 16  # global 128-row key tiles per batch
OWN = 8  # key tiles per core
QT = 4  # query tiles of 512 (in permuted local order)
SCALE = float(C) ** -0.5

_COMPILED = None
LAST_EXEC_NS = None
LAST_RESULTS = None


def _build_nc(exchange=False):
    import concourse.bass as bass
    import concourse.mybir as mybir
    import concourse.tile as tile
    from concourse import bacc
    from contextlib import ExitStack

    fp32 = mybir.dt.float32
    bf16 = mybir.dt.bfloat16

    # Bacc (not plain Bass): its compile() pipeline lowers multi-wait sync
    # info, inserts gpsimd library loads, etc. — walrus rejects the raw form.
    # detect_race_conditions=False for the exchange build: the sim's rdma race
    # detector flags the cross-core semaphore update, which is the intended
    # synchronization here (wait_ge on a peer-incremented semaphore).
    nc = bacc.Bacc(
        "TRN2",
        target_bir_lowering=False,
        debug=False,
        num_devices=8,
        detect_race_conditions=not exchange,
    )
    # Per-core inputs (host-permuted): xT columns = [my 8 key-tiles | peer 8],
    # each tile 128 consecutive global rows.  With exchange=True only the own
    # half is loaded; peer qT arrives via core-to-core remote DMA.
    xT = nc.declare_dram_parameter("xT", [C, 1024 if exchange else T], fp32, isOutput=False)
    wqk = nc.declare_dram_parameter("wqk", [C, 128], fp32, isOutput=False)
    wv = nc.declare_dram_parameter("wv", [C, H], fp32, isOutput=False)
    # Stair masks, one per (qt, si in 0..1) = slots (2qt, 2qt+1); local-q order.
    masks = nc.declare_dram_parameter("masks", [8, 128, 512], bf16, isOutput=False)
    out_ext = nc.declare_dram_parameter("out", [H + 1, T], fp32, isOutput=True)

    with ExitStack() as ctx:
        tc = ctx.enter_context(tile.TileContext(nc))
        persist = ctx.enter_context(tc.tile_pool(name="persist", bufs=1))
        weipool = ctx.enter_context(tc.tile_pool(name="wei", bufs=2))

        # ---- P0: loads (SWDGE casts fp32->bf16 in flight) ----
        T_own = 1024 if exchange else T
        xT_sb = persist.tile([128, 8, T_own], bf16, tag="xT_sb")
        for c in range(8):
            nc.gpsimd.dma_start(
                out=xT_sb[:, c, :], in_=xT[c * 128 : (c + 1) * 128, :]
            )
        wqk_sb = persist.tile([128, 8, 128], bf16, tag="wqk_sb")
        nc.gpsimd.dma_start(
            out=wqk_sb[:], in_=wqk.rearrange("(c p) j -> p c j", p=128)
        )
        wv_sb = persist.tile([128, 8, H], bf16, tag="wv_sb")
        nc.gpsimd.dma_start(out=wv_sb[:], in_=wv.rearrange("(c p) j -> p c j", p=128))
        mask_sb = persist.tile([128, 8, 512], bf16, tag="mask_sb")
        nc.gpsimd.dma_start(out=mask_sb[:], in_=masks.rearrange("s p y -> p s y"))

        # ---- P1+P2: projections, psum -> sbuf (bf16) ----
        # qT_all local query order per qt: [own(2qt), own(2qt+1), peer(2qt), peer(2qt+1)]
        qT_all = persist.tile([64, T], bf16, tag="qT_all")
        kT_own = persist.tile([64, 1024], bf16, tag="kT_own")
        vT_own = persist.tile([64, 1024], bf16, tag="vT_own")
        import concourse.bass as bass_mod

        def strided_copy(dst_tile, dst_off, src_ap):
            # copy 4 chunks of 256 cols: src chunks at 256*i, dst at 512*i + dst_off
            src = bass_mod.AP(
                tensor=src_ap.tensor,
                offset=src_ap.offset,
                ap=[src_ap.ap[0], [256, 4], [1, 256]],
            )
            d = dst_tile[:, dst_off : dst_off + 1]  # establish tensor/offset
            dst = bass_mod.AP(
                tensor=d.tensor,
                offset=d.offset,
                ap=[d.ap[0], [512, 4], [1, 256]],
            )
            nc.vector.tensor_copy(dst, src)

        if exchange:
            ex_send = persist.tile([128, 512], bf16, tag="ex_send")
            ex_recv = persist.tile([128, 512], bf16, tag="ex_recv")
            rsem = ctx.enter_context(nc.semaphore("rsem"))
            lsem = ctx.enter_context(nc.semaphore("lsem"))

        with tc.tile_pool(name="ps_proj", bufs=2, space="PSUM") as ps_proj:
            # qk over my own columns: out rows 0:64 = qT(own), 64:128 = kT(own)
            qk_ps = ps_proj.tile([128, 1024], fp32, tag="proj", name="qk_ps")
            for c in range(8):
                for n in range(2):
                    nc.tensor.matmul(
                        out=qk_ps[:, n * 512 : (n + 1) * 512],
                        lhsT=wqk_sb[:, c, :],
                        rhs=xT_sb[:, c, n * 512 : (n + 1) * 512],
                        start=(c == 0),
                        stop=(c == 7),
                    )
            strided_copy(qT_all, 0, qk_ps[0:64, :])
            nc.vector.tensor_copy(kT_own[:], qk_ps[64:128, :])

            if exchange:
                # pack my qT [64,1024] -> [128,512] and swap with pair partner
                nc.vector.tensor_copy(ex_send[0:64, :], qk_ps[0:64, 0:512])
                nc.vector.tensor_copy(ex_send[64:128, :], qk_ps[0:64, 512:1024])
            else:
                # q over peer columns
                qp_ps = ps_proj.tile([128, 1024], fp32, tag="proj", name="qp_ps")
                for c in range(8):
                    for n in range(2):
                        nc.tensor.matmul(
                            out=qp_ps[0:64, n * 512 : (n + 1) * 512],
                            lhsT=wqk_sb[:, c, 0:64],
                            rhs=xT_sb[:, c, 1024 + n * 512 : 1024 + (n + 1) * 512],
                            start=(c == 0),
                            stop=(c == 7),
                        )
                strided_copy(qT_all, 256, qp_ps[0:64, :])

            # v over my own columns
            vo_ps = ps_proj.tile([128, 1024], fp32, tag="proj", name="vo_ps")
            for c in range(8):
                for n in range(2):
                    nc.tensor.matmul(
                        out=vo_ps[0:64, n * 512 : (n + 1) * 512],
                        lhsT=wv_sb[:, c, :],
                        rhs=xT_sb[:, c, n * 512 : (n + 1) * 512],
                        start=(c == 0),
                        stop=(c == 7),
                    )
            nc.vector.tensor_copy(vT_own[:], vo_ps[0:64, :])

        if exchange:
            # swap qT halves with the pair partner (tpb XOR 1) over remote DMA,
            # then scatter peer columns into qT_all.  All on gpsimd so the
            # wait_ge -> copies ordering is plain program order.
            def unpack(dst_off, src_rows):
                d = qT_all[:, dst_off : dst_off + 1]
                dst = bass_mod.AP(
                    tensor=d.tensor, offset=d.offset, ap=[d.ap[0], [512, 2], [1, 256]]
                )
                s = ex_recv[src_rows * 64 : src_rows * 64 + 64, :]
                src = bass_mod.AP(
                    tensor=s.tensor, offset=s.offset, ap=[s.ap[0], [256, 2], [1, 256]]
                )
                nc.vector.tensor_copy(dst, src)

            with tc.tile_critical():
                # clear BEFORE our trigger: the peer's update cannot arrive
                # until after its own (symmetric) trigger, so clearing here
                # cannot wipe it; also makes the NEFF re-executable.
                # (Bacc.compile inserts the remote_dma gpsimd library load.)
                nc.gpsimd.sem_clear(rsem)
                nc.gpsimd.sem_clear(lsem)
                nc.gpsimd.remote_dma_broadcast(
                    out_ap=ex_recv[:],
                    in_ap=ex_send[:],
                    remote_sem=rsem,
                    local_sem=lsem,
                    rdests=[(0, 1)] + [None] * 7,
                )
                nc.gpsimd.trigger_dma(count=1)
                nc.vector.wait_ge(rsem, 2)
                unpack(256, 0)
                unpack(256 + 1024, 1)

        # ---- P3: v row-layout tiles with ones column ----
        # PE-mode transpose (sbuf->psum via identity), not DMA transpose —
        # the xbar transpose path hung on hardware here.
        from concourse.masks import make_identity

        v_sb = persist.tile([128, 8, H + 1], bf16, tag="v_sb")
        ident = persist.tile([128, 128], bf16, tag="ident")
        make_identity(nc, ident[:])
        with tc.tile_pool(name="ps_vt", bufs=2, space="PSUM") as ps_vt:
            for s in range(8):
                nc.vector.memset(v_sb[:, s, H : H + 1], 1.0)
                vt_ps = ps_vt.tile([128, H], bf16, tag="vt", name="vt_ps")
                nc.tensor.transpose(
                    vt_ps[:], vT_own[:, s * 128 : (s + 1) * 128], ident[0:64, 0:64]
                )
                nc.vector.tensor_copy(v_sb[:, s, 0:H], vt_ps[:])

        # ---- P4: attention (scores^T -> exp -> stair mask -> PV) ----
        out_sb = persist.tile([H + 1, T], fp32, tag="out_sb")
        ps_sc4 = ctx.enter_context(tc.tile_pool(name="ps_sc4", bufs=1, space="PSUM"))
        ps_sc2 = ctx.enter_context(tc.tile_pool(name="ps_sc2", bufs=1, space="PSUM"))
        ps_pv = ctx.enter_context(tc.tile_pool(name="ps_pv", bufs=2, space="PSUM"))

        def do_scores(ps, ps_off, qt, s):
            nc.tensor.matmul(
                out=ps[:, ps_off : ps_off + 512],
                lhsT=kT_own[:, s * 128 : (s + 1) * 128],
                rhs=qT_all[:, qt * 512 : (qt + 1) * 512],
                start=True,
                stop=True,
                skip_group_check=True,
            )

        for qt in range(QT):
            nkt = 2 * qt + 2
            wei = weipool.tile([128, 4096], bf16, tag="wei")
            # groups of score slots sharing one psum tile + one exp instruction
            groups = []
            s0 = 0
            while s0 < nkt:
                g = min(4 if nkt - s0 >= 4 else 2, nkt - s0)
                groups.append((s0, g))
                s0 += g
            for g0, glen in groups:
                if glen > 2:
                    ps = ps_sc4.tile([128, 2048], fp32, tag="sc4", name="sc4")
                else:
                    ps = ps_sc2.tile([128, 1024], fp32, tag="sc2", name="sc2")
                for i in range(glen):
                    do_scores(ps, i * 512, qt, g0 + i)
                nc.scalar.activation(
                    out=wei[:, g0 * 512 : (g0 + glen) * 512],
                    in_=ps[:, 0 : glen * 512],
                    func=mybir.ActivationFunctionType.Exp,
                    scale=SCALE,
                )
            # stair masks on the last two slots (2qt, 2qt+1)
            for si in range(2):
                slot = 2 * qt + si
                nc.vector.tensor_mul(
                    out=wei[:, slot * 512 : (slot + 1) * 512],
                    in0=wei[:, slot * 512 : (slot + 1) * 512],
                    in1=mask_sb[:, slot, :],
                )
            # PV accumulate over slots
            pv = ps_pv.tile([H + 1, 512], fp32, tag="pv")
            for s in range(nkt):
                nc.tensor.matmul(
                    out=pv[:],
                    lhsT=v_sb[:, s, :],
                    rhs=wei[:, s * 512 : (s + 1) * 512],
                    start=(s == 0),
                    stop=(s == nkt - 1),
                    skip_group_check=True,
                )
            nc.vector.tensor_copy(out_sb[:, qt * 512 : (qt + 1) * 512], pv[:])

        # ---- P5: store partial sums ----
        nc.sync.dma_start(out=out_ext[:], in_=out_sb[:])

    nc.compile()
    return nc


def _local_q_perm(p):
    """global query index for each local column (length T)."""
    perm = np.empty(T, dtype=np.int64)
    for qt in range(QT):
        blk_kts = [4 * qt + p, 4 * qt + 2 + p, 4 * qt + (1 - p), 4 * qt + 2 + (1 - p)]
        for i, kt in enumerate(blk_kts):
            lo = qt * 512 + i * 128
            perm[lo : lo + 128] = np.arange(kt * 128, kt * 128 + 128)
    return perm


def _build_masks(p):
    """[8,128,512] bf16: slot (qt, si) masks score block of own key-tile
    own_kts[2qt+si] vs the local-order query tile qt."""
    own_kts = [2 * m + p for m in range(8)]
    perm = _local_q_perm(p)
    masks = np.zeros((8, 128, 512), dtype=np.float32)
    for qt in range(QT):
        qg = perm[qt * 512 : (qt + 1) * 512]  # global query index per local col
        for si in range(2):
            kt = own_kts[2 * qt + si]
            keys = np.arange(kt * 128, kt * 128 + 128)
            masks[2 * qt + si] = (keys[:, None] <= qg[None, :]).astype(np.float32)
    return masks.astype(ml_dtypes.bfloat16)


def _make_in_maps(x, Wq, Wk, Wv, exchange=False):
    wqk = np.ascontiguousarray(np.concatenate([Wq, Wk], axis=1))
    wv = np.ascontiguousarray(Wv)
    in_maps = []
    for c in range(8):
        b, p = c // 2, c % 2
        own_kts = [2 * m + p for m in range(8)]
        peer_kts = [2 * m + (1 - p) for m in range(8)]
        kts = own_kts if exchange else own_kts + peer_kts
        rows = np.concatenate([np.arange(kt * 128, kt * 128 + 128) for kt in kts])
        xT_perm = np.ascontiguousarray(x[b][rows].T)  # [C, 1024 or T]
        in_maps.append(
            {"xT": xT_perm, "wqk": wqk, "wv": wv, "masks": _build_masks(p)}
        )
    return in_maps


def _combine(per_core_out):
    """per_core_out: list of 8 arrays [H+1, T] (local query order) -> [B,T,H]."""
    out = np.empty((B, T, H), dtype=np.float32)
    for b in range(B):
        S = None
        for p in range(2):
            P_local = np.asarray(per_core_out[2 * b + p], dtype=np.float32)
            perm = _local_q_perm(p)
            P_glob = np.empty_like(P_local)
            P_glob[:, perm] = P_local
            S = P_glob if S is None else S + P_glob
        out[b] = (S[0:H, :] / S[H : H + 1, :]).T
    return out


EXCHANGE = os.environ.get("BASS_KERNEL_EXCHANGE", "0") == "1"


def kernel(x, Wq, Wk, Wv):
    global _COMPILED, LAST_EXEC_NS, LAST_RESULTS
    from concourse.bass_utils import run_bass_kernel_spmd

    x = np.ascontiguousarray(np.asarray(x, dtype=np.float32))
    Wq = np.asarray(Wq, dtype=np.float32)
    Wk = np.asarray(Wk, dtype=np.float32)
    Wv = np.asarray(Wv, dtype=np.float32)

    if _COMPILED is None:
        _COMPILED = _build_nc(exchange=EXCHANGE)
    nc = _COMPILED

    in_maps = _make_in_maps(x, Wq, Wk, Wv, exchange=EXCHANGE)
    trace = os.environ.get("BASS_KERNEL_TRACE", "0") == "1"
    res = run_bass_kernel_spmd(nc, in_maps, core_ids=list(range(8)), trace=trace)
    LAST_EXEC_NS = getattr(res, "exec_time_ns", None)
    LAST_RESULTS = res
    return _combine([res.results[c]["out"] for c in range(8)])



# revision 3
# speedup vs baseline: 1.2421x; 1.2421x over previous
"""Single-head causal attention (B=4, T=2048, C=1024, H=64) on 8 TRN2 NeuronCores.

Sharding: batch b -> core pair (2b, 2b+1). Within a pair, core parity p owns
the interleaved 128-row key tiles {2m+p, m=0..7}.  Each core projects q for
ALL 2048 queries (own + peer x columns, shipped bf16 host-cast) and k,v for
its own 1024 keys, computes causal scores^T -> exp -> stair mask -> wei@[v|1]
partial sums for all queries vs its own keys.  Host adds the pair's partials
and normalizes (denominator = ones-column of the augmented v matmul).

Local query-column order per 512-col tile qt (chunks of 128):
  [own tile 2qt+1 | peer tile 2qt+1 | own tile 2qt | peer tile 2qt]
With own tiles ascending (global 2j+p), slot j (key tile j) vs tile qt is:
  full for j < 2qt on all 4 chunks;
  slot 2qt   : width 512, chunks [full, full, stair, X];
  slot 2qt+1 : width 256, chunks [stair, X]  (chunks 2,3 entirely causal-zero);
where X = full (p=0) or zero (p=1).  Both masked regions use the SAME
host-supplied [128, 256] mask  Mx = [tril-stair | X] -> one tiny mask tensor,
identical instruction stream on every core.

Perf structure (vs v0 baseline, 76.7us):
 - x shipped bf16 (host cast): 4 MB/core instead of 8 MB fp32.
 - chunk-pipelined DMA (4x1MB HWDGE on sync queue) overlapping projections.
 - PE pre-warm matmuls during the first DMA (HAM clock-gate release).
 - scores row-tiled 2x: even/odd key slots run concurrently in the top and
   bottom 64-row halves of the PE array (K=64 each).
 - causal width trimming: slot 2qt+1 computes only 256 cols.
 - exp grouped per slot-pair on Scalar ACT; mask muls on gpsimd; PSUM
   evacuations on Vector.
"""

import os
import sys

sys.path.insert(0, "/opt/trn_rl_repo")

import numpy as np
import ml_dtypes

B, T, C, H = 4, 2048, 1024, 64
QT = 4  # 512-col query tiles
SCALE = float(C) ** -0.5

_COMPILED = None
LAST_EXEC_NS = None
LAST_RESULTS = None


def _build_nc():
    import concourse.bass as bass_mod
    import concourse.mybir as mybir
    import concourse.tile as tile
    from concourse import bacc
    from concourse.masks import make_identity
    from contextlib import ExitStack

    fp32 = mybir.dt.float32
    bf16 = mybir.dt.bfloat16

    nc = bacc.Bacc(
        "TRN2",
        target_bir_lowering=False,
        debug=False,
        num_devices=8,
        detect_race_conditions=True,
    )
    # [partition, C-chunk, own 1024 | peer 1024] bf16 (host pre-cast/permuted)
    xT = nc.declare_dram_parameter("xT", [128, 8, 2048], bf16, isOutput=False)
    wqk = nc.declare_dram_parameter("wqk", [128, 8, 128], bf16, isOutput=False)
    wv = nc.declare_dram_parameter("wv", [128, 8, 64], bf16, isOutput=False)
    mask = nc.declare_dram_parameter("mask", [128, 256], bf16, isOutput=False)
    out_ext = nc.declare_dram_parameter("out", [H + 1, T], fp32, isOutput=True)

    with ExitStack() as ctx:
        tc = ctx.enter_context(tile.TileContext(nc))
        persist = ctx.enter_context(tc.tile_pool(name="persist", bufs=1))
        weipool = ctx.enter_context(tc.tile_pool(name="wei", bufs=2))
        outpool = ctx.enter_context(tc.tile_pool(name="outp", bufs=2))

        xT_sb = persist.tile([128, 8, 2048], bf16, tag="xT_sb")
        wqk_sb = persist.tile([128, 8, 128], bf16, tag="wqk_sb")
        wv_sb = persist.tile([128, 8, 64], bf16, tag="wv_sb")
        mask_sb = persist.tile([128, 256], bf16, tag="mask_sb")
        q_sb = persist.tile([128, T], bf16, tag="q_sb")
        k_sb = persist.tile([128, 1024], bf16, tag="k_sb")
        vT_sb = persist.tile([64, 1024], bf16, tag="vT_sb")
        v_sb = persist.tile([128, 8, H + 1], bf16, tag="v_sb")
        ident = persist.tile([128, 128], bf16, tag="ident")
        scratch = persist.tile([128, 512], bf16, tag="scratch")  # uninit, warmup rhs
        warm_tok = persist.tile([1, 8], fp32, tag="warm_tok")

        # ---- loads: weights/mask on the scalar HWDGE ring, x on sync ----
        nc.scalar.dma_start(out=wqk_sb[:], in_=wqk[:])
        nc.scalar.dma_start(out=wv_sb[:], in_=wv[:])
        nc.scalar.dma_start(out=mask_sb[:], in_=mask[:])
        for g in range(4):
            nc.sync.dma_start(
                out=xT_sb[:, 2 * g : 2 * g + 2, :], in_=xT[:, 2 * g : 2 * g + 2, :]
            )

        make_identity(nc, ident[:])
        nc.vector.memset(scratch[:], 0.0)

        # ---- PE pre-warm while the first x chunk is in flight ----
        with tc.tile_pool(name="ps_warm", bufs=1, space="PSUM") as ps_warm:
            wps = ps_warm.tile([128, 512], fp32, tag="warm", name="warm_ps")
            for i in range(6):
                nc.tensor.matmul(
                    out=wps[:],
                    lhsT=ident[:],
                    rhs=scratch[:],
                    start=(i == 0),
                    stop=(i == 5),
                    skip_group_check=True,
                )
            nc.vector.tensor_copy(warm_tok[0:1, 0:8], wps[0:1, 0:8])

        # ---- projections, chunk-pipelined with the x DMAs ----
        # qk_ps: q_own (parts 0:64) | k_own (64:128); qp_ps: q_peer; vo_ps: v_own
        with tc.tile_pool(name="ps_proj", bufs=1, space="PSUM") as ps_proj:
            qk_ps = ps_proj.tile([128, 1024], fp32, tag="qk", name="qk_ps")
            qp_ps = ps_proj.tile([64, 1024], fp32, tag="qp", name="qp_ps")
            vo_ps = ps_proj.tile([64, 1024], fp32, tag="vo", name="vo_ps")
            for c in range(8):
                st, sp = (c == 0), (c == 7)
                for n in range(2):
                    nc.tensor.matmul(
                        out=qk_ps[:, n * 512 : (n + 1) * 512],
                        lhsT=wqk_sb[:, c, :],
                        rhs=xT_sb[:, c, n * 512 : (n + 1) * 512],
                        start=st,
                        stop=sp,
                        skip_group_check=True,
                    )
                for n in range(2):
                    nc.tensor.matmul(
                        out=qp_ps[:, n * 512 : (n + 1) * 512],
                        lhsT=wqk_sb[:, c, 0:64],
                        rhs=xT_sb[:, c, 1024 + n * 512 : 1024 + (n + 1) * 512],
                        start=st,
                        stop=sp,
                        skip_group_check=True,
                    )
                for n in range(2):
                    nc.tensor.matmul(
                        out=vo_ps[:, n * 512 : (n + 1) * 512],
                        lhsT=wv_sb[:, c, :],
                        rhs=xT_sb[:, c, n * 512 : (n + 1) * 512],
                        start=st,
                        stop=sp,
                        skip_group_check=True,
                    )

            # ---- evacuations ----
            def strided4(src_base, src_coloff, dst_base, dst_coloff, width=128):
                """copy 4 chunks of `width` cols: src cols src_coloff+256*i,
                dst cols dst_coloff+512*i, partition rows from given APs."""
                s = src_base[:, src_coloff : src_coloff + 1]
                src = bass_mod.AP(
                    tensor=s.tensor, offset=s.offset, ap=[s.ap[0], [256, 4], [1, width]]
                )
                d = dst_base[:, dst_coloff : dst_coloff + 1]
                dst = bass_mod.AP(
                    tensor=d.tensor, offset=d.offset, ap=[d.ap[0], [512, 4], [1, width]]
                )
                nc.vector.tensor_copy(dst, src)

            # q top half: own odd tiles -> chunk0 (col 0), own even -> chunk2 (256)
            strided4(qk_ps[0:64, :], 128, q_sb[0:64, :], 0)
            strided4(qk_ps[0:64, :], 0, q_sb[0:64, :], 256)
            # peer odd -> chunk1 (128), peer even -> chunk3 (384)
            strided4(qp_ps[:, :], 128, q_sb[0:64, :], 128)
            strided4(qp_ps[:, :], 0, q_sb[0:64, :], 384)
            # bottom half duplicate (gpsimd, SBUF->SBUF)
            nc.gpsimd.tensor_copy(q_sb[64:128, :], q_sb[0:64, :])
            # k: psum parts 64:128 -> k_sb rows 64:128, dup to rows 0:64
            nc.vector.tensor_copy(k_sb[64:128, :], qk_ps[64:128, :])
            nc.gpsimd.tensor_copy(k_sb[0:64, :], k_sb[64:128, :])
            # v
            nc.vector.tensor_copy(vT_sb[:], vo_ps[:])

        # ---- v row-layout via PE transpose + ones column ----
        nc.gpsimd.memset(v_sb[:, :, H : H + 1], 1.0)
        with tc.tile_pool(name="ps_vt", bufs=2, space="PSUM") as ps_vt:
            for j in range(8):
                vt_ps = ps_vt.tile([128, H], bf16, tag="vt", name="vt_ps")
                nc.tensor.transpose(
                    vt_ps[:], vT_sb[:, j * 128 : (j + 1) * 128], ident[0:64, 0:64]
                )
                nc.vector.tensor_copy(v_sb[:, j, 0:H], vt_ps[:])

        # ---- attention ----
        ps_pair = ctx.enter_context(tc.tile_pool(name="ps_pair", bufs=3, space="PSUM"))
        ps_pv = ctx.enter_context(tc.tile_pool(name="ps_pv", bufs=2, space="PSUM"))

        for qt in range(QT):
            wei = weipool.tile([128, 4096], bf16, tag="wei")
            for i in range(qt + 1):
                je, jo = 2 * i, 2 * i + 1
                wo = 256 if jo == 2 * qt + 1 else 512
                pair_ps = ps_pair.tile([128, 1024], fp32, tag="pair", name="pair_ps")
                # even slot: top 64 rows of the PE array; odd slot: bottom 64
                nc.tensor.matmul(
                    out=pair_ps[:, 0:512],
                    lhsT=k_sb[0:64, je * 128 : (je + 1) * 128],
                    rhs=q_sb[0:64, qt * 512 : qt * 512 + 512],
                    start=True,
                    stop=True,
                    skip_group_check=True,
                )
                nc.tensor.matmul(
                    out=pair_ps[:, 512 : 512 + wo],
                    lhsT=k_sb[64:128, jo * 128 : (jo + 1) * 128],
                    rhs=q_sb[64:128, qt * 512 : qt * 512 + wo],
                    start=True,
                    stop=True,
                    skip_group_check=True,
                )
                nc.scalar.activation(
                    out=wei[:, je * 512 : je * 512 + 512 + wo],
                    in_=pair_ps[:, 0 : 512 + wo],
                    func=mybir.ActivationFunctionType.Exp,
                    scale=SCALE,
                )
            # stair/X masks on the last two slots (gpsimd; SBUF bf16)
            nc.gpsimd.tensor_mul(
                out=wei[:, 2 * qt * 512 + 256 : 2 * qt * 512 + 512],
                in0=wei[:, 2 * qt * 512 + 256 : 2 * qt * 512 + 512],
                in1=mask_sb[:],
            )
            nc.gpsimd.tensor_mul(
                out=wei[:, (2 * qt + 1) * 512 : (2 * qt + 1) * 512 + 256],
                in0=wei[:, (2 * qt + 1) * 512 : (2 * qt + 1) * 512 + 256],
                in1=mask_sb[:],
            )
            # PV accumulate over slots
            pv = ps_pv.tile([H + 1, 512], fp32, tag="pv", name="pv_ps")
            nslots = 2 * qt + 2
            for j in range(nslots):
                w = 256 if j == nslots - 1 else 512
                nc.tensor.matmul(
                    out=pv[:, 0:w],
                    lhsT=v_sb[:, j, :],
                    rhs=wei[:, j * 512 : j * 512 + w],
                    start=(j == 0),
                    stop=(j == nslots - 1),
                    skip_group_check=True,
                )
            out_t = outpool.tile([H + 1, 512], fp32, tag="out_t")
            nc.vector.tensor_copy(out_t[:], pv[:])
            nc.sync.dma_start(out=out_ext[:, qt * 512 : (qt + 1) * 512], in_=out_t[:])

    nc.compile()
    return nc


def _own_rows(p):
    """global x/key row indices owned by parity p, own tiles ascending."""
    return np.concatenate(
        [np.arange((2 * j + p) * 128, (2 * j + p) * 128 + 128) for j in range(8)]
    )


def _local_q_perm(p):
    """global query index for each local output column (length T)."""
    perm = np.empty(T, dtype=np.int64)
    for qt in range(QT):
        tiles = [4 * qt + 2 + p, 4 * qt + 3 - p, 4 * qt + p, 4 * qt + 1 - p]
        for ci, g in enumerate(tiles):
            lo = qt * 512 + ci * 128
            perm[lo : lo + 128] = np.arange(g * 128, g * 128 + 128)
    return perm


def _make_in_maps(x, Wq, Wk, Wv):
    bf = ml_dtypes.bfloat16
    wqk = np.concatenate([Wq, Wk], axis=1)  # [C, 128]
    wqk_pre = np.ascontiguousarray(
        wqk.reshape(8, 128, 128).transpose(1, 0, 2).astype(bf)
    )
    wv_pre = np.ascontiguousarray(Wv.reshape(8, 128, 64).transpose(1, 0, 2).astype(bf))
    tri = (np.arange(128)[:, None] <= np.arange(128)[None, :]).astype(np.float32)
    in_maps = []
    for c in range(8):
        b, p = c // 2, c % 2
        rows = np.concatenate([_own_rows(p), _own_rows(1 - p)])
        xT_pre = np.ascontiguousarray(
            x[b][rows].T.reshape(8, 128, 2048).transpose(1, 0, 2).astype(bf)
        )
        X = np.ones((128, 128), np.float32) if p == 0 else np.zeros((128, 128), np.float32)
        mask_pre = np.ascontiguousarray(np.concatenate([tri, X], axis=1).astype(bf))
        in_maps.append(
            {"xT": xT_pre, "wqk": wqk_pre, "wv": wv_pre, "mask": mask_pre}
        )
    return in_maps


def _combine(per_core_out):
    """list of 8 arrays [H+1, T] (local query order) -> [B, T, H]."""
    out = np.empty((B, T, H), dtype=np.float32)
    for b in range(B):
        S = None
        for p in range(2):
            P_local = np.asarray(per_core_out[2 * b + p], dtype=np.float32)
            perm = _local_q_perm(p)
            P_glob = np.empty_like(P_local)
            P_glob[:, perm] = P_local
            S = P_glob if S is None else S + P_glob
        out[b] = (S[0:H, :] / S[H : H + 1, :]).T
    return out


def kernel(x, Wq, Wk, Wv):
    global _COMPILED, LAST_EXEC_NS, LAST_RESULTS
    from concourse.bass_utils import run_bass_kernel_spmd

    x = np.ascontiguousarray(np.asarray(x, dtype=np.float32))
    Wq = np.asarray(Wq, dtype=np.float32)
    Wk = np.asarray(Wk, dtype=np.float32)
    Wv = np.asarray(Wv, dtype=np.float32)

    if _COMPILED is None:
        _COMPILED = _build_nc()
    nc = _COMPILED

    in_maps = _make_in_maps(x, Wq, Wk, Wv)
    trace = os.environ.get("BASS_KERNEL_TRACE", "0") == "1"
    res = run_bass_kernel_spmd(nc, in_maps, core_ids=list(range(8)), trace=trace)
    LAST_EXEC_NS = getattr(res, "exec_time_ns", None)
    LAST_RESULTS = res
    return _combine([res.results[c]["out"] for c in range(8)])


# revision 12
# speedup vs baseline: 1.3002x; 1.0468x over previous
"""Single-head causal attention (B=4, T=2048, C=1024, H=64) on 8 TRN2 NeuronCores.

Sharding: batch b -> core pair (2b, 2b+1); core parity p owns interleaved
128-row key tiles {2m+p}.  Each core projects q for ALL 2048 queries and k,v
for its own 1024 keys, computes causal scores^T -> exp -> stair mask ->
wei@[v|1] partials for all queries vs its own keys.  Host adds pair partials
and normalizes (denominator = ones-column of the augmented v matmul).

Data layout (all bf16, host pre-cast):
 - x columns per core: [own tiles ascending | peer tiles ascending].
 - q is produced DIRECTLY in the scores layout by two col-tiled (M=64)
   projections with 4D strided rhs APs over x:
     q_sb[0:64,  qt*256+j]  = q of chunk pair [own(2qt+1) | peer(2qt+1)]
     q_sb[64:128, qt*256+j] = q of chunk pair [own(2qt)   | peer(2qt)]
 - k is projected with weights [wk|wk] -> duplicated in both partition
   halves for free (scores row-tiling needs lhsT in each 64-row half).
 - scores slot j (own key tile j) vs query tile qt: top-half mm covers local
   chunks {0,1}, bottom-half mm covers chunks {2,3}; the two run concurrently
   in the two 64-row halves of the PE array (K=64 row tiling).
 - causal trimming: slot 2qt+1 skips its bottom mm (chunks 2,3 are zero);
   both stair regions multiply the SAME host mask Mx=[tril|X] (X=1 for p=0,
   0 for p=1), so the instruction stream is core-invariant.

Local query-column order per 512-col tile qt (output):
  [own(2qt+1) | peer(2qt+1) | own(2qt) | peer(2qt)]
"""

import os
import sys

sys.path.insert(0, "/opt/trn_rl_repo")

import numpy as np
import ml_dtypes

B, T, C, H = 4, 2048, 1024, 64
QT = 4
SCALE = float(C) ** -0.5

_COMPILED = None
LAST_EXEC_NS = None
LAST_RESULTS = None


def _build_nc():
    import concourse.bass as bass_mod
    import concourse.mybir as mybir
    import concourse.tile as tile
    from concourse import bacc
    from concourse.masks import make_identity
    from contextlib import ExitStack

    fp32 = mybir.dt.float32
    bf16 = mybir.dt.bfloat16

    nc = bacc.Bacc(
        "TRN2",
        target_bir_lowering=False,
        debug=False,
        num_devices=8,
        detect_race_conditions=True,
    )
    xT = nc.declare_dram_parameter("xT", [128, 8, 2048], bf16, isOutput=False)
    # [wq|wk] (own-stream lhsT) and [wq|wq] (peer-stream lhsT)
    wqkk = nc.declare_dram_parameter("wqkk", [128, 8, 256], bf16, isOutput=False)
    wv = nc.declare_dram_parameter("wv", [128, 8, 64], bf16, isOutput=False)
    mask = nc.declare_dram_parameter("mask", [128, 256], bf16, isOutput=False)
    out_ext = nc.declare_dram_parameter("out", [H + 1, T], fp32, isOutput=True)

    with ExitStack() as ctx:
        tc = ctx.enter_context(tile.TileContext(nc))
        persist = ctx.enter_context(tc.tile_pool(name="persist", bufs=1))
        weipool = ctx.enter_context(tc.tile_pool(name="wei", bufs=2))
        outpool = ctx.enter_context(tc.tile_pool(name="outp", bufs=2))

        xT_sb = persist.tile([128, 8, 2048], bf16, tag="xT_sb")
        wqkk_sb = persist.tile([128, 8, 256], bf16, tag="wqkk_sb")
        wv_sb = persist.tile([128, 8, 64], bf16, tag="wv_sb")
        mask_sb = persist.tile([128, 256], bf16, tag="mask_sb")
        q_sb = persist.tile([128, T], bf16, tag="q_sb")
        k_sb = persist.tile([128, 1024], bf16, tag="k_sb")
        vT_sb = persist.tile([64, 1024], bf16, tag="vT_sb")
        v_sb = persist.tile([128, 8, H + 1], bf16, tag="v_sb")
        ident = persist.tile([128, 128], bf16, tag="ident")
        scratch = persist.tile([128, 512], bf16, tag="scratch")
        warm_tok = persist.tile([1, 8], fp32, tag="warm_tok")

        # ---- loads: gpsimd SWDGE issues immediately (no ucode-load stall) ----
        nc.gpsimd.dma_start(out=wqkk_sb[:], in_=wqkk[:])
        nc.gpsimd.dma_start(out=wv_sb[:], in_=wv[:])
        nc.gpsimd.dma_start(out=mask_sb[:], in_=mask[:])
        nc.gpsimd.dma_start(out=xT_sb[:, 0:2, :], in_=xT[:, 0:2, :])
        nc.gpsimd.dma_start(out=xT_sb[:, 2:4, :], in_=xT[:, 2:4, :])
        # later chunks on the sync HWDGE ring (its ~4.7us ucode delay is hidden)
        nc.sync.dma_start(out=xT_sb[:, 4:6, :], in_=xT[:, 4:6, :])
        nc.sync.dma_start(out=xT_sb[:, 6:8, :], in_=xT[:, 6:8, :])

        nc.gpsimd.memset(scratch[:], 0.0)
        make_identity(nc, ident[:])

        # ---- PE pre-warm while the first x chunk is in flight ----
        with tc.tile_pool(name="ps_warm", bufs=1, space="PSUM") as ps_warm:
            wps = ps_warm.tile([128, 512], fp32, tag="warm", name="warm_ps")
            for i in range(3):
                nc.tensor.matmul(
                    out=wps[:],
                    lhsT=ident[:],
                    rhs=scratch[:],
                    start=(i == 0),
                    stop=(i == 2),
                    skip_group_check=True,
                )
            nc.vector.tensor_copy(warm_tok[0:1, 0:8], wps[0:1, 0:8])

        # ---- projections, chunk-pipelined with the x DMAs ----
        # qk_ps: q_own (parts 0:64) | k_own (64:128)  over own x columns
        # qp_ps: q_peer duplicated in both halves ([wq|wq]) over peer columns
        with tc.tile_pool(name="ps_proj", bufs=1, space="PSUM") as ps_proj:
            qk_ps = ps_proj.tile([128, 1024], fp32, tag="qk", name="qk_ps")
            qp_ps = ps_proj.tile([128, 1024], fp32, tag="qp", name="qp_ps")
            vv_ps = ps_proj.tile([64, 1024], fp32, tag="vv", name="vv_ps")
            for c in range(8):
                st, sp = (c == 0), (c == 7)
                xc = xT_sb[:, c, :]
                for n in range(2):
                    nc.tensor.matmul(
                        out=qk_ps[:, n * 512 : (n + 1) * 512],
                        lhsT=wqkk_sb[:, c, 0:128],
                        rhs=xc[:, n * 512 : (n + 1) * 512],
                        start=st,
                        stop=sp,
                        skip_group_check=True,
                    )
                for n in range(2):
                    nc.tensor.matmul(
                        out=qp_ps[:, n * 512 : (n + 1) * 512],
                        lhsT=wqkk_sb[:, c, 128:256],
                        rhs=xc[:, 1024 + n * 512 : 1024 + (n + 1) * 512],
                        start=st,
                        stop=sp,
                        skip_group_check=True,
                    )
                for n in range(2):
                    nc.tensor.matmul(
                        out=vv_ps[:, n * 512 : (n + 1) * 512],
                        lhsT=wv_sb[:, c, :],
                        rhs=xc[:, n * 512 : (n + 1) * 512],
                        start=st,
                        stop=sp,
                        skip_group_check=True,
                    )

            # ---- evacuations ----
            def strided4(src_base, src_coloff, dst_base, dst_coloff):
                """4 chunks of 128 cols: src cols src_coloff+256*i (tile i of
                the asc projection), dst cols dst_coloff+512*i (query tile i)."""
                s = src_base[:, src_coloff : src_coloff + 1]
                src = bass_mod.AP(
                    tensor=s.tensor, offset=s.offset, ap=[s.ap[0], [256, 4], [1, 128]]
                )
                d = dst_base[:, dst_coloff : dst_coloff + 1]
                dst = bass_mod.AP(
                    tensor=d.tensor, offset=d.offset, ap=[d.ap[0], [512, 4], [1, 128]]
                )
                nc.vector.tensor_copy(dst, src)

            # q_sb top: chunk order [own odd | peer odd | own even | peer even]
            strided4(qk_ps[0:64, :], 128, q_sb[0:64, :], 0)
            strided4(qp_ps[0:64, :], 128, q_sb[0:64, :], 128)
            strided4(qk_ps[0:64, :], 0, q_sb[0:64, :], 256)
            strided4(qp_ps[0:64, :], 0, q_sb[0:64, :], 384)
            # k evac (no shift) on vector; v evac
            nc.vector.tensor_copy(k_sb[64:128, :], qk_ps[64:128, :])
            nc.vector.tensor_copy(vT_sb[:], vv_ps[:])

        # partition-half duplicates via SBUF->SBUF DMA (off the engines)
        nc.sync.dma_start(out=q_sb[64:128, :], in_=q_sb[0:64, :])
        nc.sync.dma_start(out=k_sb[0:64, :], in_=k_sb[64:128, :])

        # ---- attention (+ v transposes interleaved on the PE) ----
        nc.gpsimd.memset(v_sb[:, :, H : H + 1], 1.0)
        ps_pair = ctx.enter_context(tc.tile_pool(name="ps_pair", bufs=3, space="PSUM"))
        ps_pv = ctx.enter_context(tc.tile_pool(name="ps_pv", bufs=2, space="PSUM"))

        for j in range(2):  # first two v tiles (needed by qt=0) before scores
            vt_ps = ps_pair.tile([128, H], bf16, tag="pair", name="vt_ps")
            nc.tensor.transpose(
                vt_ps[:, 0:H], vT_sb[:, j * 128 : (j + 1) * 128], ident[0:64, 0:64]
            )
            nc.vector.tensor_copy(v_sb[:, j, 0:H], vt_ps[:, 0:H])

        for qt in range(QT):
            wei = weipool.tile([128, 4096], bf16, tag="wei")
            for i in range(qt + 1):
                je, jo = 2 * i, 2 * i + 1
                last = jo == 2 * qt + 1
                wo = 256 if last else 512
                pair_ps = ps_pair.tile([128, 1024], fp32, tag="pair", name="pair_ps")
                # even slot -> bank 0 (cols 0:512), odd slot -> bank 1: the
                # two row-tiled mms run concurrently in different PSUM banks.
                nc.tensor.matmul(
                    out=pair_ps[:, 0:512],
                    lhsT=k_sb[0:64, je * 128 : (je + 1) * 128],
                    rhs=q_sb[0:64, qt * 512 : qt * 512 + 512],
                    start=True,
                    stop=True,
                    skip_group_check=True,
                )
                nc.tensor.matmul(
                    out=pair_ps[:, 512 : 512 + wo],
                    lhsT=k_sb[64:128, jo * 128 : (jo + 1) * 128],
                    rhs=q_sb[64:128, qt * 512 : qt * 512 + wo],
                    start=True,
                    stop=True,
                    skip_group_check=True,
                )
                nc.scalar.activation(
                    out=wei[:, je * 512 : je * 512 + 512 + wo],
                    in_=pair_ps[:, 0 : 512 + wo],
                    func=mybir.ActivationFunctionType.Exp,
                    scale=SCALE,
                )
            # stair/X masks on the last two slots (vector; bf16 SBUF)
            nc.vector.tensor_mul(
                out=wei[:, 2 * qt * 512 + 256 : 2 * qt * 512 + 512],
                in0=wei[:, 2 * qt * 512 + 256 : 2 * qt * 512 + 512],
                in1=mask_sb[:],
            )
            nc.vector.tensor_mul(
                out=wei[:, (2 * qt + 1) * 512 : (2 * qt + 1) * 512 + 256],
                in0=wei[:, (2 * qt + 1) * 512 : (2 * qt + 1) * 512 + 256],
                in1=mask_sb[:],
            )
            # PV accumulate over slots
            pv = ps_pv.tile([H + 1, 512], fp32, tag="pv", name="pv_ps")
            nslots = 2 * qt + 2
            for j in range(nslots):
                w = 256 if j == nslots - 1 else 512
                nc.tensor.matmul(
                    out=pv[:, 0:w],
                    lhsT=v_sb[:, j, :],
                    rhs=wei[:, j * 512 : j * 512 + w],
                    start=(j == 0),
                    stop=(j == nslots - 1),
                    skip_group_check=True,
                )
            # v transposes for the next qt (slots 2qt+2, 2qt+3)
            if qt < QT - 1:
                for j in (2 * qt + 2, 2 * qt + 3):
                    vt_ps = ps_pair.tile([128, H], bf16, tag="pair", name="vt_ps")
                    nc.tensor.transpose(
                        vt_ps[:, 0:H],
                        vT_sb[:, j * 128 : (j + 1) * 128],
                        ident[0:64, 0:64],
                    )
                    nc.vector.tensor_copy(v_sb[:, j, 0:H], vt_ps[:, 0:H])
            out_t = outpool.tile([H + 1, 512], fp32, tag="out_t")
            nc.vector.tensor_copy(out_t[:], pv[:])
            nc.sync.dma_start(out=out_ext[:, qt * 512 : (qt + 1) * 512], in_=out_t[:])

    nc.compile()
    return nc


def _own_rows(p):
    return np.concatenate(
        [np.arange((2 * j + p) * 128, (2 * j + p) * 128 + 128) for j in range(8)]
    )


def _local_q_perm(p):
    perm = np.empty(T, dtype=np.int64)
    for qt in range(QT):
        tiles = [4 * qt + 2 + p, 4 * qt + 3 - p, 4 * qt + p, 4 * qt + 1 - p]
        for ci, g in enumerate(tiles):
            lo = qt * 512 + ci * 128
            perm[lo : lo + 128] = np.arange(g * 128, g * 128 + 128)
    return perm


def _make_in_maps(x, Wq, Wk, Wv):
    bf = ml_dtypes.bfloat16
    wqkk = np.concatenate([Wq, Wk, Wq, Wq], axis=1)  # [C, 256]
    wqkk_pre = np.ascontiguousarray(
        wqkk.reshape(8, 128, 256).transpose(1, 0, 2).astype(bf)
    )
    wv_pre = np.ascontiguousarray(Wv.reshape(8, 128, 64).transpose(1, 0, 2).astype(bf))
    tri = (np.arange(128)[:, None] <= np.arange(128)[None, :]).astype(np.float32)
    in_maps = []
    for c in range(8):
        b, p = c // 2, c % 2
        rows = np.concatenate([_own_rows(p), _own_rows(1 - p)])
        xT_pre = np.ascontiguousarray(
            x[b][rows].T.reshape(8, 128, 2048).transpose(1, 0, 2).astype(bf)
        )
        X = np.ones((128, 128), np.float32) if p == 0 else np.zeros((128, 128), np.float32)
        mask_pre = np.ascontiguousarray(np.concatenate([tri, X], axis=1).astype(bf))
        in_maps.append(
            {"xT": xT_pre, "wqkk": wqkk_pre, "wv": wv_pre, "mask": mask_pre}
        )
    return in_maps


def _combine(per_core_out):
    out = np.empty((B, T, H), dtype=np.float32)
    for b in range(B):
        S = None
        for p in range(2):
            P_local = np.asarray(per_core_out[2 * b + p], dtype=np.float32)
            perm = _local_q_perm(p)
            P_glob = np.empty_like(P_local)
            P_glob[:, perm] = P_local
            S = P_glob if S is None else S + P_glob
        out[b] = (S[0:H, :] / S[H : H + 1, :]).T
    return out


def kernel(x, Wq, Wk, Wv):
    global _COMPILED, LAST_EXEC_NS, LAST_RESULTS
    from concourse.bass_utils import run_bass_kernel_spmd

    x = np.ascontiguousarray(np.asarray(x, dtype=np.float32))
    Wq = np.asarray(Wq, dtype=np.float32)
    Wk = np.asarray(Wk, dtype=np.float32)
    Wv = np.asarray(Wv, dtype=np.float32)

    if _COMPILED is None:
        _COMPILED = _build_nc()
    nc = _COMPILED

    in_maps = _make_in_maps(x, Wq, Wk, Wv)
    trace = os.environ.get("BASS_KERNEL_TRACE", "0") == "1"
    res = run_bass_kernel_spmd(nc, in_maps, core_ids=list(range(8)), trace=trace)
    LAST_EXEC_NS = getattr(res, "exec_time_ns", None)
    LAST_RESULTS = res
    return _combine([res.results[c]["out"] for c in range(8)])


# revision 20
# speedup vs baseline: 1.4318x; 1.1012x over previous
"""Single-head causal attention (B=4, T=2048, C=1024, H=64) on 8 TRN2 NeuronCores.

Sharding: batch b -> core pair (2b, 2b+1); core parity p owns interleaved
128-row key tiles {2m+p}.  Each core projects q for ALL 2048 queries and k,v
for its own 1024 keys, computes causal scores^T -> exp -> stair mask ->
wei@[v|1] partials for all queries vs its own keys.  Host adds pair partials
and normalizes (denominator = ones-column of the augmented v matmul).

Data layout (all bf16, host pre-cast):
 - x columns per core: [own tiles ascending | peer tiles ascending].
 - q is produced DIRECTLY in the scores layout by two col-tiled (M=64)
   projections with 4D strided rhs APs over x:
     q_sb[0:64,  qt*256+j]  = q of chunk pair [own(2qt+1) | peer(2qt+1)]
     q_sb[64:128, qt*256+j] = q of chunk pair [own(2qt)   | peer(2qt)]
 - k is projected with weights [wk|wk] -> duplicated in both partition
   halves for free (scores row-tiling needs lhsT in each 64-row half).
 - scores slot j (own key tile j) vs query tile qt: top-half mm covers local
   chunks {0,1}, bottom-half mm covers chunks {2,3}; the two run concurrently
   in the two 64-row halves of the PE array (K=64 row tiling).
 - causal trimming: slot 2qt+1 skips its bottom mm (chunks 2,3 are zero);
   both stair regions multiply the SAME host mask Mx=[tril|X] (X=1 for p=0,
   0 for p=1), so the instruction stream is core-invariant.

Local query-column order per 512-col tile qt (output):
  [own(2qt+1) | peer(2qt+1) | own(2qt) | peer(2qt)]
"""

import os
import sys

sys.path.insert(0, "/opt/trn_rl_repo")

import numpy as np
import ml_dtypes

B, T, C, H = 4, 2048, 1024, 64
QT = 4
SCALE = float(C) ** -0.5

_COMPILED = None
LAST_EXEC_NS = None
LAST_RESULTS = None


def _build_nc():
    import concourse.bass as bass_mod
    import concourse.mybir as mybir
    import concourse.tile as tile
    from concourse import bacc
    from concourse.masks import make_identity
    from contextlib import ExitStack

    fp32 = mybir.dt.float32
    bf16 = mybir.dt.bfloat16

    nc = bacc.Bacc(
        "TRN2",
        target_bir_lowering=False,
        debug=False,
        num_devices=8,
        detect_race_conditions=True,
    )
    xT = nc.declare_dram_parameter("xT", [128, 8, 2048], bf16, isOutput=False)
    # [wq|wk] (own-stream lhsT) and [wq|wq] (peer-stream lhsT)
    wqkk = nc.declare_dram_parameter("wqkk", [128, 8, 256], bf16, isOutput=False)
    wv = nc.declare_dram_parameter("wv", [128, 8, 64], bf16, isOutput=False)
    mask = nc.declare_dram_parameter("mask", [128, 256], bf16, isOutput=False)
    out_ext = nc.declare_dram_parameter("out", [H + 1, T], fp32, isOutput=True)

    with ExitStack() as ctx:
        tc = ctx.enter_context(tile.TileContext(nc))
        persist = ctx.enter_context(tc.tile_pool(name="persist", bufs=1))
        weipool = ctx.enter_context(tc.tile_pool(name="wei", bufs=2))
        outpool = ctx.enter_context(tc.tile_pool(name="outp", bufs=2))

        xT_sb = persist.tile([128, 8, 2048], bf16, tag="xT_sb")
        wqkk_sb = persist.tile([128, 8, 256], bf16, tag="wqkk_sb")
        wv_sb = persist.tile([128, 8, 64], bf16, tag="wv_sb")
        mask_sb = persist.tile([128, 256], bf16, tag="mask_sb")
        q_sb = persist.tile([128, T], bf16, tag="q_sb")
        k_sb = persist.tile([128, 1024], bf16, tag="k_sb")
        vT_sb = persist.tile([64, 1024], bf16, tag="vT_sb")
        v_sb = persist.tile([128, 8, H + 1], bf16, tag="v_sb")
        ident = persist.tile([128, 128], bf16, tag="ident")
        scratch = persist.tile([128, 512], bf16, tag="scratch")
        warm_tok = persist.tile([1, 8], fp32, tag="warm_tok")

        # ---- loads: x split over both HWDGE rings (sync + scalar) so chunk
        # pairs arrive progressively; weights ride in front (tiny).
        nc.gpsimd.memset(scratch[:], 0.0)
        nc.sync.dma_start(out=wqkk_sb[:], in_=wqkk[:])
        nc.sync.dma_start(out=xT_sb[:, 0:2, :], in_=xT[:, 0:2, :])
        nc.sync.dma_start(out=xT_sb[:, 2:4, :], in_=xT[:, 2:4, :])
        nc.scalar.dma_start(out=wv_sb[:], in_=wv[:])
        nc.scalar.dma_start(out=mask_sb[:], in_=mask[:])
        nc.scalar.dma_start(out=xT_sb[:, 4:6, :], in_=xT[:, 4:6, :])
        nc.scalar.dma_start(out=xT_sb[:, 6:8, :], in_=xT[:, 6:8, :])
        make_identity(nc, ident[:])

        # ---- PE pre-warm while the first x chunks are in flight; also
        # preload the scalar engine's Exp spline table off the critical path.
        act_tok = persist.tile([1, 8], bf16, tag="act_tok")
        with tc.tile_pool(name="ps_warm", bufs=1, space="PSUM") as ps_warm:
            wps = ps_warm.tile([128, 512], fp32, tag="warm", name="warm_ps")
            for i in range(6):
                nc.tensor.matmul(
                    out=wps[:],
                    lhsT=scratch[:, 0:128],
                    rhs=scratch[:],
                    start=(i == 0),
                    stop=(i == 5),
                    skip_group_check=True,
                )
            nc.vector.tensor_copy(warm_tok[0:1, 0:8], wps[0:1, 0:8])
        nc.scalar.activation(
            out=act_tok[0:1, 0:8],
            in_=scratch[0:1, 0:8],
            func=mybir.ActivationFunctionType.Exp,
        )

        # ---- projections, chunk-pipelined with the x DMAs ----
        # qk_ps: q_own (parts 0:64) | k_own (64:128)  over own x columns
        # qp_ps: q_peer duplicated in both halves ([wq|wq]) over peer columns
        with tc.tile_pool(name="ps_proj", bufs=1, space="PSUM") as ps_proj:
            qk_ps = ps_proj.tile([128, 1024], fp32, tag="qk", name="qk_ps")
            qp_ps = ps_proj.tile([128, 1024], fp32, tag="qp", name="qp_ps")
            vv_ps = ps_proj.tile([64, 1024], fp32, tag="vv", name="vv_ps")
            corder = [0, 1, 4, 5, 2, 3, 6, 7]  # expected DMA arrival order
            for ci, c in enumerate(corder):
                st, sp = (ci == 0), (ci == 7)
                xc = xT_sb[:, c, :]
                for n in range(2):
                    nc.tensor.matmul(
                        out=qk_ps[:, n * 512 : (n + 1) * 512],
                        lhsT=wqkk_sb[:, c, 0:128],
                        rhs=xc[:, n * 512 : (n + 1) * 512],
                        start=st,
                        stop=sp,
                        skip_group_check=True,
                    )
                for n in range(2):
                    nc.tensor.matmul(
                        out=qp_ps[:, n * 512 : (n + 1) * 512],
                        lhsT=wqkk_sb[:, c, 128:256],
                        rhs=xc[:, 1024 + n * 512 : 1024 + (n + 1) * 512],
                        start=st,
                        stop=sp,
                        skip_group_check=True,
                    )
                for n in range(2):
                    nc.tensor.matmul(
                        out=vv_ps[:, n * 512 : (n + 1) * 512],
                        lhsT=wv_sb[:, c, :],
                        rhs=xc[:, n * 512 : (n + 1) * 512],
                        start=st,
                        stop=sp,
                        skip_group_check=True,
                    )

            # ---- evacuations ----
            # Host ships x own/peer columns pair-swapped ([o1,o0,o3,o2,...]),
            # so psum block b holds tile o_{b^1} and the q scatter into local
            # order [own-odd | peer-odd | own-even | peer-even] is ONE
            # strided-dst copy per stream (dst block b at col 256*b).
            def scatter8(src_base, dst_base, dst_coloff):
                s = src_base[:, 0:1]
                src = bass_mod.AP(
                    tensor=s.tensor, offset=s.offset, ap=[s.ap[0], [128, 8], [1, 128]]
                )
                d = dst_base[:, dst_coloff : dst_coloff + 1]
                dst = bass_mod.AP(
                    tensor=d.tensor, offset=d.offset, ap=[d.ap[0], [256, 8], [1, 128]]
                )
                nc.vector.tensor_copy(dst, src)

            # k first (shifted down: k native in TOP half like q)
            nc.vector.tensor_copy(k_sb[0:64, :], qk_ps[64:128, :])
            scatter8(qk_ps[0:64, :], q_sb[0:64, :], 0)
            scatter8(qp_ps[0:64, :], q_sb[0:64, :], 128)
            # partition-half duplicates (fast bf16 SBUF->SBUF on DVE)
            nc.vector.tensor_copy(k_sb[64:128, :], k_sb[0:64, :])
            nc.vector.tensor_copy(q_sb[64:128, :], q_sb[0:64, :])
            # v evac on scalar (ACT Copy) to keep vector free
            nc.scalar.copy(vT_sb[:], vv_ps[:])
        nc.scalar.activation(
            out=act_tok[0:1, 0:8],
            in_=scratch[0:1, 0:8],
            func=mybir.ActivationFunctionType.Exp,
        )

        # ---- attention (+ v transposes interleaved on the PE) ----
        nc.gpsimd.memset(v_sb[:, :, H : H + 1], 1.0)
        ps_pair = ctx.enter_context(tc.tile_pool(name="ps_pair", bufs=3, space="PSUM"))
        ps_pv = ctx.enter_context(tc.tile_pool(name="ps_pv", bufs=2, space="PSUM"))

        for j in range(2):  # first two v tiles (needed by qt=0) before scores
            vt_ps = ps_pair.tile([128, H], bf16, tag="pair", name="vt_ps")
            nc.tensor.transpose(
                vt_ps[:, 0:H],
                vT_sb[:, (j ^ 1) * 128 : ((j ^ 1) + 1) * 128],
                ident[0:64, 0:64],
            )
            nc.vector.tensor_copy(v_sb[:, j, 0:H], vt_ps[:, 0:H])

        for qt in range(QT):
            wei = weipool.tile([128, 4096], bf16, tag="wei")
            for i in range(qt + 1):
                je, jo = 2 * i, 2 * i + 1
                last = jo == 2 * qt + 1
                wo = 256 if last else 512
                pair_ps = ps_pair.tile([128, 1024], fp32, tag="pair", name="pair_ps")
                # even slot -> bank 0 (cols 0:512), odd slot -> bank 1: the
                # two row-tiled mms run concurrently in different PSUM banks.
                ke, ko = je ^ 1, jo ^ 1
                nc.tensor.matmul(
                    out=pair_ps[:, 0:512],
                    lhsT=k_sb[0:64, ke * 128 : (ke + 1) * 128],
                    rhs=q_sb[0:64, qt * 512 : qt * 512 + 512],
                    start=True,
                    stop=True,
                    skip_group_check=True,
                )
                nc.tensor.matmul(
                    out=pair_ps[:, 512 : 512 + wo],
                    lhsT=k_sb[64:128, ko * 128 : (ko + 1) * 128],
                    rhs=q_sb[64:128, qt * 512 : qt * 512 + wo],
                    start=True,
                    stop=True,
                    skip_group_check=True,
                )
                nc.scalar.activation(
                    out=wei[:, je * 512 : je * 512 + 512 + wo],
                    in_=pair_ps[:, 0 : 512 + wo],
                    func=mybir.ActivationFunctionType.Exp,
                    scale=SCALE,
                )
            # stair/X masks on the last two slots (vector; bf16 SBUF)
            nc.vector.tensor_mul(
                out=wei[:, 2 * qt * 512 + 256 : 2 * qt * 512 + 512],
                in0=wei[:, 2 * qt * 512 + 256 : 2 * qt * 512 + 512],
                in1=mask_sb[:],
            )
            nc.vector.tensor_mul(
                out=wei[:, (2 * qt + 1) * 512 : (2 * qt + 1) * 512 + 256],
                in0=wei[:, (2 * qt + 1) * 512 : (2 * qt + 1) * 512 + 256],
                in1=mask_sb[:],
            )
            # PV accumulate over slots
            pv = ps_pv.tile([H + 1, 512], fp32, tag="pv", name="pv_ps")
            nslots = 2 * qt + 2
            for j in range(nslots):
                w = 256 if j == nslots - 1 else 512
                nc.tensor.matmul(
                    out=pv[:, 0:w],
                    lhsT=v_sb[:, j, :],
                    rhs=wei[:, j * 512 : j * 512 + w],
                    start=(j == 0),
                    stop=(j == nslots - 1),
                    skip_group_check=True,
                )
            # v transposes for the next qt (slots 2qt+2, 2qt+3)
            if qt < QT - 1:
                for j in (2 * qt + 2, 2 * qt + 3):
                    vt_ps = ps_pair.tile([128, H], bf16, tag="pair", name="vt_ps")
                    nc.tensor.transpose(
                        vt_ps[:, 0:H],
                        vT_sb[:, (j ^ 1) * 128 : ((j ^ 1) + 1) * 128],
                        ident[0:64, 0:64],
                    )
                    nc.vector.tensor_copy(v_sb[:, j, 0:H], vt_ps[:, 0:H])
            out_t = outpool.tile([H + 1, 512], fp32, tag="out_t")
            nc.vector.tensor_copy(out_t[:], pv[:])
            nc.sync.dma_start(out=out_ext[:, qt * 512 : (qt + 1) * 512], in_=out_t[:])

    nc.compile()
    return nc


def _own_rows(p):
    """x column order for parity p: own tiles PAIR-SWAPPED [o1,o0,o3,o2,...]
    so the q psum scatters to local order with a single strided copy."""
    order = [1, 0, 3, 2, 5, 4, 7, 6]
    return np.concatenate(
        [np.arange((2 * j + p) * 128, (2 * j + p) * 128 + 128) for j in order]
    )


def _local_q_perm(p):
    perm = np.empty(T, dtype=np.int64)
    for qt in range(QT):
        tiles = [4 * qt + 2 + p, 4 * qt + 3 - p, 4 * qt + p, 4 * qt + 1 - p]
        for ci, g in enumerate(tiles):
            lo = qt * 512 + ci * 128
            perm[lo : lo + 128] = np.arange(g * 128, g * 128 + 128)
    return perm


def _make_in_maps(x, Wq, Wk, Wv):
    bf = ml_dtypes.bfloat16
    wqkk = np.concatenate([Wq, Wk, Wq, Wq], axis=1)  # [C, 256]
    wqkk_pre = np.ascontiguousarray(
        wqkk.reshape(8, 128, 256).transpose(1, 0, 2).astype(bf)
    )
    wv_pre = np.ascontiguousarray(Wv.reshape(8, 128, 64).transpose(1, 0, 2).astype(bf))
    tri = (np.arange(128)[:, None] <= np.arange(128)[None, :]).astype(np.float32)
    in_maps = []
    for c in range(8):
        b, p = c // 2, c % 2
        rows = np.concatenate([_own_rows(p), _own_rows(1 - p)])
        xT_pre = np.ascontiguousarray(
            x[b][rows].T.reshape(8, 128, 2048).transpose(1, 0, 2).astype(bf)
        )
        X = np.ones((128, 128), np.float32) if p == 0 else np.zeros((128, 128), np.float32)
        mask_pre = np.ascontiguousarray(np.concatenate([tri, X], axis=1).astype(bf))
        in_maps.append(
            {"xT": xT_pre, "wqkk": wqkk_pre, "wv": wv_pre, "mask": mask_pre}
        )
    return in_maps


def _combine(per_core_out):
    out = np.empty((B, T, H), dtype=np.float32)
    for b in range(B):
        S = None
        for p in range(2):
            P_local = np.asarray(per_core_out[2 * b + p], dtype=np.float32)
            perm = _local_q_perm(p)
            P_glob = np.empty_like(P_local)
            P_glob[:, perm] = P_local
            S = P_glob if S is None else S + P_glob
        out[b] = (S[0:H, :] / S[H : H + 1, :]).T
    return out


def kernel(x, Wq, Wk, Wv):
    global _COMPILED, LAST_EXEC_NS, LAST_RESULTS
    from concourse.bass_utils import run_bass_kernel_spmd

    x = np.ascontiguousarray(np.asarray(x, dtype=np.float32))
    Wq = np.asarray(Wq, dtype=np.float32)
    Wk = np.asarray(Wk, dtype=np.float32)
    Wv = np.asarray(Wv, dtype=np.float32)

    if _COMPILED is None:
        _COMPILED = _build_nc()
    nc = _COMPILED

    in_maps = _make_in_maps(x, Wq, Wk, Wv)
    trace = os.environ.get("BASS_KERNEL_TRACE", "0") == "1"
    res = run_bass_kernel_spmd(nc, in_maps, core_ids=list(range(8)), trace=trace)
    LAST_EXEC_NS = getattr(res, "exec_time_ns", None)
    LAST_RESULTS = res
    return _combine([res.results[c]["out"] for c in range(8)])
